# revision 2
# baseline (speedup 1.0000x reference)
"""TRN2 Bass kernel for nn_Attention_89584427860152.

General attention: e = (q @ Wa^T) @ kv^T; attn = softmax(e); ctx = attn @ kv.
Returns (ctx, attn). mask is all-False by construction (spec fill=zeros) and
is ignored.

Sharding: batch dim (b=8) -> one batch per NeuronCore, Wa replicated.

Per-core plan (QL=KVL=2048, QD=KVD=512):
  phase 0: load q/kv/Wa; PE-transpose into contraction layouts (qT, kvT, WaT
           as float32r, which the PE runs at full rate with ~tf32 precision);
           mm1: q_projT = WaT.T @ qT   (f32r)
  phase 1 (per 128-row q-tile):
           e = q_projT.T @ kvT into PSUM (f32r, full fp32 accumulate)
           rowmax (DVE) -> exp(e - max) + row-sum in one ACT pass
           attn_norm (f32) -> HBM;  attn bf16 -> xbar DMA transpose ->
           ctx = attnT.T @ kv_bf16 (bf16 matmul), scaled by 1/rowsum at
           PSUM evacuation.
"""

import sys

if "/opt/trn_rl_repo" not in sys.path:
    sys.path.insert(0, "/opt/trn_rl_repo")

from contextlib import ExitStack

import numpy as np

import concourse.bass as bass
import concourse.tile as tile
from concourse import bacc, mybir
from concourse.bass_utils import run_bass_kernel_spmd
from concourse.masks import make_identity

F32 = mybir.dt.float32
F32R = mybir.dt.float32r
BF16 = mybir.dt.bfloat16

B, QL, KVL, QD, KVD = 8, 2048, 2048, 512, 512
NQT = QL // 128      # 16 q row-tiles
NKT = KVL // 128     # 16 kv row-tiles
NCT = KVD // 128     # 4 contraction tiles (kv feature dim)
NDT = QD // 128      # 4 contraction tiles (q feature dim)
NKC = KVL // 512     # 4 key column chunks of 512


def _emit(tc, ctx_ap, attn_ap, q_ap, kv_ap, wa_ap):
    nc = tc.nc
    es = ExitStack()
    with es:
        # ---------- persistent SBUF ----------
        persist = es.enter_context(tc.tile_pool(name="persist", bufs=1))
        ident = persist.tile([128, 128], F32, tag="ident")
        make_identity(nc, ident[:])

        waT = [persist.tile([128, 512], F32R, tag=f"waT{dt}", name=f"waT{dt}") for dt in range(NDT)]
        qT = [persist.tile([128, QL], F32R, tag=f"qT{dt}", name=f"qT{dt}") for dt in range(NDT)]
        kvT = [persist.tile([128, KVL], F32R, tag=f"kvT{ct}", name=f"kvT{ct}") for ct in range(NCT)]
        qpT = [persist.tile([128, QL], F32R, tag=f"qpT{ct}", name=f"qpT{ct}") for ct in range(NCT)]
        kv_bf = [persist.tile([128, 512], BF16, tag=f"kvbf{kt}", name=f"kvbf{kt}") for kt in range(NKT)]

        # ---------- phase 0: loads + transposes + mm1 ----------
        with (
            tc.tile_pool(name="p0in", bufs=3) as p0in,
            tc.tile_pool(name="p0ps", bufs=4, space="PSUM") as p0ps,
            tc.tile_pool(name="p0mm", bufs=2, space="PSUM") as p0mm,
        ):
            # Wa: one 1 MiB DMA, [p, ct, d]
            wa_sb = p0in.tile([128, NCT, 512], F32, tag="wa")
            nc.sync.dma_start(wa_sb[:], wa_ap.rearrange("(ct p) d -> p ct d", p=128))
            for ct in range(NCT):
                for dt in range(NDT):
                    tp = p0ps.tile([128, 128], F32, tag="tp")
                    nc.tensor.transpose(
                        tp[:], wa_sb[:, ct, dt * 128 : (dt + 1) * 128], ident[:]
                    )
                    nc.vector.tensor_copy(waT[dt][:, ct * 128 : (ct + 1) * 128], tp[:])

            # q: 4 DMAs of 1 MiB, each covers 4 row-tiles; transpose + mm1
            # grouped per 512-wide q chunk so mm1 follows its transposes.
            for g in range(4):
                q_sb = p0in.tile([128, 4, 512], F32, tag="qin")
                nc.sync.dma_start(
                    q_sb[:],
                    q_ap.rearrange("(g i p) c -> g p i c", g=4, p=128)[g],
                )
                for i in range(4):
                    qt_idx = g * 4 + i
                    for dt in range(NDT):
                        tp = p0ps.tile([128, 128], F32, tag="tp")
                        nc.tensor.transpose(
                            tp[:], q_sb[:, i, dt * 128 : (dt + 1) * 128], ident[:]
                        )
                        nc.vector.tensor_copy(
                            qT[dt][:, qt_idx * 128 : (qt_idx + 1) * 128], tp[:]
                        )
                # mm1 for this q chunk: q_projT[:, g*512:+512]
                for ct in range(NCT):
                    mm = p0mm.tile([128, 512], F32, tag="mm1")
                    for dt in range(NDT):
                        nc.tensor.matmul(
                            mm[:],
                            waT[dt][:, ct * 128 : (ct + 1) * 128],
                            qT[dt][:, g * 512 : (g + 1) * 512],
                            start=(dt == 0),
                            stop=(dt == NDT - 1),
                        )
                    nc.vector.tensor_copy(qpT[ct][:, g * 512 : (g + 1) * 512], mm[:])

            # kv: 4 DMAs of 1 MiB; transpose to kvT + cast to bf16
            for g in range(4):
                kv_sb = p0in.tile([128, 4, 512], F32, tag="kvin")
                nc.sync.dma_start(
                    kv_sb[:],
                    kv_ap.rearrange("(g i p) c -> g p i c", g=4, p=128)[g],
                )
                for i in range(4):
                    kt_idx = g * 4 + i
                    nc.vector.tensor_copy(kv_bf[kt_idx][:], kv_sb[:, i, :])
                    for ct in range(NCT):
                        tp = p0ps.tile([128, 128], F32, tag="tp")
                        nc.tensor.transpose(
                            tp[:], kv_sb[:, i, ct * 128 : (ct + 1) * 128], ident[:]
                        )
                        nc.vector.tensor_copy(
                            kvT[ct][:, kt_idx * 128 : (kt_idx + 1) * 128], tp[:]
                        )

        # ---------- phase 1: per q-tile attention ----------
        with (
            tc.tile_pool(name="p1sb", bufs=2) as p1sb,
            tc.tile_pool(name="p1st", bufs=3) as p1st,
            tc.tile_pool(name="eps", bufs=3, space="PSUM") as eps,
            tc.tile_pool(name="cps", bufs=2, space="PSUM") as cps,
        ):
            for qt in range(NQT):
                qs = slice(qt * 128, (qt + 1) * 128)
                # e tile: two [128, 1024] halves (2 PSUM banks each)
                e_h = []
                for h in range(2):
                    eh = eps.tile([128, 1024], F32, tag="e")
                    for kc2 in range(2):
                        kc = 2 * h + kc2
                        for ct in range(NCT):
                            nc.tensor.matmul(
                                eh[:, kc2 * 512 : (kc2 + 1) * 512],
                                qpT[ct][:, qs],
                                kvT[ct][:, kc * 512 : (kc + 1) * 512],
                                start=(ct == 0),
                                stop=(ct == NCT - 1),
                            )
                    e_h.append(eh)

                # row stats: max over both halves, negated
                pmax = p1st.tile([128, 2], F32, tag="pmax")
                for h in range(2):
                    nc.vector.tensor_reduce(
                        pmax[:, h : h + 1], e_h[h][:],
                        axis=mybir.AxisListType.X, op=mybir.AluOpType.max,
                    )
                negmax = p1st.tile([128, 1], F32, tag="negmax")
                nc.vector.tensor_reduce(
                    negmax[:], pmax[:], axis=mybir.AxisListType.X,
                    op=mybir.AluOpType.max, negate=True,
                )

                # exp(e - max) with fused row-sum; unnormalized attn in f32
                a_un = p1sb.tile([128, 2048], F32, tag="a_un")
                ssum = p1st.tile([128, 2], F32, tag="ssum")
                for h in range(2):
                    nc.scalar.activation(
                        a_un[:, h * 1024 : (h + 1) * 1024], e_h[h][:],
                        mybir.ActivationFunctionType.Exp,
                        bias=negmax[:], scale=1.0,
                        accum_out=ssum[:, h : h + 1],
                    )
                tsum = p1st.tile([128, 1], F32, tag="tsum")
                nc.vector.tensor_add(tsum[:], ssum[:, 0:1], ssum[:, 1:2])
                rsum = p1st.tile([128, 1], F32, tag="rsum")
                nc.vector.reciprocal(rsum[:], tsum[:])

                # normalized f32 attn -> HBM
                a_nm = p1sb.tile([128, 2048], F32, tag="a_nm")
                nc.vector.tensor_scalar_mul(a_nm[:], a_un[:], rsum[:])
                nc.sync.dma_start(attn_ap[qs, :], a_nm[:])

                # bf16 unnormalized attn -> xbar transpose -> attnT
                a_bf = p1sb.tile([128, 2048], BF16, tag="a_bf")
                nc.vector.tensor_copy(a_bf[:], a_un[:])
                a_tr = p1sb.tile([128, 2048], BF16, tag="a_tr")
                for kt in range(NKT):
                    ks = slice(kt * 128, (kt + 1) * 128)
                    nc.sync.dma_start(a_tr[:, ks], a_bf[:, ks], transpose=True)

                # ctx = attnT.T @ kv (bf16), scale by 1/rowsum at evacuation
                cx = cps.tile([128, 512], F32, tag="cx")
                for kt in range(NKT):
                    ks = slice(kt * 128, (kt + 1) * 128)
                    nc.tensor.matmul(
                        cx[:], a_tr[:, ks], kv_bf[kt][:],
                        start=(kt == 0), stop=(kt == NKT - 1),
                    )
                c_sb = p1sb.tile([128, 512], F32, tag="c_sb")
                nc.vector.tensor_scalar_mul(c_sb[:], cx[:], rsum[:])
                nc.sync.dma_start(ctx_ap[qs, :], c_sb[:])


def _build():
    nc = bacc.Bacc("TRN2", target_bir_lowering=False, debug=False)
    q = nc.dram_tensor("q_seq", [QL, QD], F32, kind="ExternalInput").ap()
    kv = nc.dram_tensor("kv_seq", [KVL, KVD], F32, kind="ExternalInput").ap()
    wa = nc.dram_tensor("Wa", [KVD, QD], F32, kind="ExternalInput").ap()
    ctx_t = nc.dram_tensor("ctx", [QL, KVD], F32, kind="ExternalOutput").ap()
    attn_t = nc.dram_tensor("attn", [QL, KVL], F32, kind="ExternalOutput").ap()
    with tile.TileContext(nc) as tc:
        _emit(tc, ctx_t, attn_t, q, kv, wa)
    nc.compile()
    return nc


_CACHE = {}


def _get_nc():
    if "nc" not in _CACHE:
        _CACHE["nc"] = _build()
    return _CACHE["nc"]


def kernel(q_seq, kv_seq, Wa, mask=None, _trace=False, **_ignored):
    nc = _get_nc()
    wa = np.ascontiguousarray(Wa, dtype=np.float32)
    in_maps = [
        {
            "q_seq": np.ascontiguousarray(q_seq[b], dtype=np.float32),
            "kv_seq": np.ascontiguousarray(kv_seq[b], dtype=np.float32),
            "Wa": wa,
        }
        for b in range(B)
    ]
    res = run_bass_kernel_spmd(
        nc, in_maps, core_ids=list(range(B)), trace=_trace
    )
    ctx = np.stack([res.results[b]["ctx"] for b in range(B)])
    attn = np.stack([res.results[b]["attn"] for b in range(B)])
    if _trace:
        kernel.last_results = res
    return ctx, attn


# revision 5
# speedup vs baseline: 2.1609x; 2.1609x over previous
"""TRN2 Bass kernel for nn_Attention_89584427860152.

General attention: e = (q @ Wa^T) @ kv^T; attn = softmax(e); ctx = attn @ kv.
Returns (ctx, attn). mask is all-False by construction (spec fill=zeros) and
is ignored.

Sharding: batch dim (b=8) -> one batch per NeuronCore, Wa replicated.

Per-core plan (QL=KVL=2048, QD=KVD=512):
  phase 0: load q/kv/Wa; PE-transpose into contraction layouts (qT, kvT, WaT
           as float32r, which the PE runs at full rate with ~tf32 precision);
           mm1: q_projT = WaT.T @ qT   (f32r)
  phase 1 (per 128-row q-tile):
           e = q_projT.T @ kvT into PSUM (f32r, full fp32 accumulate)
           rowmax (DVE) -> exp(e - max) + row-sum in one ACT pass
           attn_norm (f32) -> HBM;  attn bf16 -> xbar DMA transpose ->
           ctx = attnT.T @ kv_bf16 (bf16 matmul), scaled by 1/rowsum at
           PSUM evacuation.
"""

import sys

if "/opt/trn_rl_repo" not in sys.path:
    sys.path.insert(0, "/opt/trn_rl_repo")

from contextlib import ExitStack

import numpy as np

import concourse.bass as bass
import concourse.tile as tile
from concourse import bacc, mybir
from concourse.bass_utils import run_bass_kernel_spmd
from concourse.masks import make_identity

F32 = mybir.dt.float32
F32R = mybir.dt.float32r
BF16 = mybir.dt.bfloat16

B, QL, KVL, QD, KVD = 8, 2048, 2048, 512, 512
NQT = QL // 128      # 16 q row-tiles
NKT = KVL // 128     # 16 kv row-tiles
NCT = KVD // 128     # 4 contraction tiles (kv feature dim)
NDT = QD // 128      # 4 contraction tiles (q feature dim)
NKC = KVL // 512     # 4 key column chunks of 512


def _emit(tc, ctx_ap, attn_ap, q_ap, kv_ap, wa_ap):
    nc = tc.nc
    es = ExitStack()
    with es:
        # ---------- persistent SBUF ----------
        persist = es.enter_context(tc.tile_pool(name="persist", bufs=1))
        ident = persist.tile([128, 128], F32, tag="ident")
        make_identity(nc, ident[:])
        ident_bf = persist.tile([128, 128], BF16, tag="ident_bf")
        nc.vector.tensor_copy(ident_bf[:], ident[:])

        waT = [persist.tile([128, 512], F32R, tag=f"waT{dt}", name=f"waT{dt}") for dt in range(NDT)]
        qT = [persist.tile([128, QL], F32R, tag=f"qT{dt}", name=f"qT{dt}") for dt in range(NDT)]
        kvT = [persist.tile([128, KVL], F32R, tag=f"kvT{ct}", name=f"kvT{ct}") for ct in range(NCT)]
        qpT = [persist.tile([128, QL], F32R, tag=f"qpT{ct}", name=f"qpT{ct}") for ct in range(NCT)]
        kv_bf = [persist.tile([128, 512], BF16, tag=f"kvbf{kt}", name=f"kvbf{kt}") for kt in range(NKT)]

        # ---------- phase 0: loads + transposes + mm1 ----------
        with (
            tc.tile_pool(name="p0in", bufs=3) as p0in,
            tc.tile_pool(name="p0ps", bufs=4, space="PSUM") as p0ps,
            tc.tile_pool(name="p0mm", bufs=2, space="PSUM") as p0mm,
        ):
            # Wa: one 1 MiB DMA, [p, ct, d]
            wa_sb = p0in.tile([128, NCT, 512], F32, tag="wa")
            nc.sync.dma_start(wa_sb[:], wa_ap.rearrange("(ct p) d -> p ct d", p=128))
            for ct in range(NCT):
                for dt in range(NDT):
                    tp = p0ps.tile([128, 128], F32, tag="tp")
                    nc.tensor.transpose(
                        tp[:], wa_sb[:, ct, dt * 128 : (dt + 1) * 128], ident[:]
                    )
                    nc.vector.tensor_copy(waT[dt][:, ct * 128 : (ct + 1) * 128], tp[:])

            # q: 4 DMAs of 1 MiB, each covers 4 row-tiles; transpose + mm1
            # grouped per 512-wide q chunk so mm1 follows its transposes.
            for g in range(4):
                q_sb = p0in.tile([128, 4, 512], F32, tag="qin")
                nc.sync.dma_start(
                    q_sb[:],
                    q_ap.rearrange("(g i p) c -> g p i c", g=4, p=128)[g],
                )
                for i in range(4):
                    qt_idx = g * 4 + i
                    for dt in range(NDT):
                        tp = p0ps.tile([128, 128], F32, tag="tp")
                        nc.tensor.transpose(
                            tp[:], q_sb[:, i, dt * 128 : (dt + 1) * 128], ident[:]
                        )
                        nc.vector.tensor_copy(
                            qT[dt][:, qt_idx * 128 : (qt_idx + 1) * 128], tp[:]
                        )
                # mm1 for this q chunk: q_projT[:, g*512:+512]
                for ct in range(NCT):
                    mm = p0mm.tile([128, 512], F32, tag="mm1")
                    for dt in range(NDT):
                        nc.tensor.matmul(
                            mm[:],
                            waT[dt][:, ct * 128 : (ct + 1) * 128],
                            qT[dt][:, g * 512 : (g + 1) * 512],
                            start=(dt == 0),
                            stop=(dt == NDT - 1),
                        )
                    nc.vector.tensor_copy(qpT[ct][:, g * 512 : (g + 1) * 512], mm[:])

            # kv: 4 DMAs of 1 MiB; transpose to kvT + cast to bf16
            for g in range(4):
                kv_sb = p0in.tile([128, 4, 512], F32, tag="kvin")
                nc.sync.dma_start(
                    kv_sb[:],
                    kv_ap.rearrange("(g i p) c -> g p i c", g=4, p=128)[g],
                )
                for i in range(4):
                    kt_idx = g * 4 + i
                    nc.vector.tensor_copy(kv_bf[kt_idx][:], kv_sb[:, i, :])
                    for ct in range(NCT):
                        tp = p0ps.tile([128, 128], F32, tag="tp")
                        nc.tensor.transpose(
                            tp[:], kv_sb[:, i, ct * 128 : (ct + 1) * 128], ident[:]
                        )
                        nc.vector.tensor_copy(
                            kvT[ct][:, kt_idx * 128 : (kt_idx + 1) * 128], tp[:]
                        )

        # ---------- phase 1: per q-tile attention ----------
        with (
            tc.tile_pool(name="p1sb", bufs=2) as p1sb,
            tc.tile_pool(name="p1st", bufs=3) as p1st,
            tc.tile_pool(name="eps", bufs=2, space="PSUM") as eps,
            tc.tile_pool(name="cps", bufs=2, space="PSUM") as cps,
            tc.tile_pool(name="tps", bufs=2, space="PSUM") as tps,
        ):
            for qt in range(NQT):
                qs = slice(qt * 128, (qt + 1) * 128)
                # e tile: two [128, 1024] halves (2 PSUM banks each)
                e_h = []
                for h in range(2):
                    eh = eps.tile([128, 1024], F32, tag="e")
                    for kc2 in range(2):
                        kc = 2 * h + kc2
                        for ct in range(NCT):
                            nc.tensor.matmul(
                                eh[:, kc2 * 512 : (kc2 + 1) * 512],
                                qpT[ct][:, qs],
                                kvT[ct][:, kc * 512 : (kc + 1) * 512],
                                start=(ct == 0),
                                stop=(ct == NCT - 1),
                            )
                    e_h.append(eh)

                # row stats: max over both halves, negated
                pmax = p1st.tile([128, 2], F32, tag="pmax")
                for h in range(2):
                    nc.vector.tensor_reduce(
                        pmax[:, h : h + 1], e_h[h][:],
                        axis=mybir.AxisListType.X, op=mybir.AluOpType.max,
                    )
                negmax = p1st.tile([128, 1], F32, tag="negmax")
                nc.vector.tensor_reduce(
                    negmax[:], pmax[:], axis=mybir.AxisListType.X,
                    op=mybir.AluOpType.max, negate=True,
                )

                # exp(e - max) with fused row-sum; unnormalized attn in f32
                a_un = p1sb.tile([128, 2048], F32, tag="a_un")
                ssum = p1st.tile([128, 2], F32, tag="ssum")
                for h in range(2):
                    nc.scalar.activation(
                        a_un[:, h * 1024 : (h + 1) * 1024], e_h[h][:],
                        mybir.ActivationFunctionType.Exp,
                        bias=negmax[:], scale=1.0,
                        accum_out=ssum[:, h : h + 1],
                    )
                tsum = p1st.tile([128, 1], F32, tag="tsum")
                nc.vector.tensor_add(tsum[:], ssum[:, 0:1], ssum[:, 1:2])
                rsum = p1st.tile([128, 1], F32, tag="rsum")
                nc.vector.reciprocal(rsum[:], tsum[:])

                # normalized f32 attn -> HBM
                a_nm = p1sb.tile([128, 2048], F32, tag="a_nm")
                nc.vector.tensor_scalar_mul(a_nm[:], a_un[:], rsum[:])
                nc.sync.dma_start(attn_ap[qs, :], a_nm[:])

                # bf16 unnormalized attn -> PE transpose -> attnT
                a_bf = p1sb.tile([128, 2048], BF16, tag="a_bf")
                nc.vector.tensor_copy(a_bf[:], a_un[:])
                a_tr = p1sb.tile([128, 2048], BF16, tag="a_tr")
                for kt in range(NKT):
                    ks = slice(kt * 128, (kt + 1) * 128)
                    tb = tps.tile([128, 128], BF16, tag="tb")
                    nc.tensor.transpose(tb[:], a_bf[:, ks], ident_bf[:])
                    nc.scalar.copy(a_tr[:, ks], tb[:])

                # ctx = attnT.T @ kv (bf16), scale by 1/rowsum at evacuation
                cx = cps.tile([128, 512], F32, tag="cx")
                for kt in range(NKT):
                    ks = slice(kt * 128, (kt + 1) * 128)
                    nc.tensor.matmul(
                        cx[:], a_tr[:, ks], kv_bf[kt][:],
                        start=(kt == 0), stop=(kt == NKT - 1),
                    )
                c_sb = p1sb.tile([128, 512], F32, tag="c_sb")
                nc.vector.tensor_scalar_mul(c_sb[:], cx[:], rsum[:])
                nc.sync.dma_start(ctx_ap[qs, :], c_sb[:])


def _build():
    nc = bacc.Bacc("TRN2", target_bir_lowering=False, debug=False)
    q = nc.dram_tensor("q_seq", [QL, QD], F32, kind="ExternalInput").ap()
    kv = nc.dram_tensor("kv_seq", [KVL, KVD], F32, kind="ExternalInput").ap()
    wa = nc.dram_tensor("Wa", [KVD, QD], F32, kind="ExternalInput").ap()
    ctx_t = nc.dram_tensor("ctx", [QL, KVD], F32, kind="ExternalOutput").ap()
    attn_t = nc.dram_tensor("attn", [QL, KVL], F32, kind="ExternalOutput").ap()
    with tile.TileContext(nc) as tc:
        _emit(tc, ctx_t, attn_t, q, kv, wa)
    nc.compile()
    return nc


_CACHE = {}


def _get_nc():
    if "nc" not in _CACHE:
        _CACHE["nc"] = _build()
    return _CACHE["nc"]


def kernel(q_seq, kv_seq, Wa, mask=None, _trace=False, **_ignored):
    nc = _get_nc()
    wa = np.ascontiguousarray(Wa, dtype=np.float32)
    in_maps = [
        {
            "q_seq": np.ascontiguousarray(q_seq[b], dtype=np.float32),
            "kv_seq": np.ascontiguousarray(kv_seq[b], dtype=np.float32),
            "Wa": wa,
        }
        for b in range(B)
    ]
    res = run_bass_kernel_spmd(
        nc, in_maps, core_ids=list(range(B)), trace=_trace
    )
    ctx = np.stack([res.results[b]["ctx"] for b in range(B)])
    attn = np.stack([res.results[b]["attn"] for b in range(B)])
    if _trace:
        kernel.last_results = res
    return ctx, attn
